# revision 1
# baseline (speedup 1.0000x reference)
"""Trainium2 Bass kernel for CausalGraphLayer (gnn message passing).

out[b,n,t,c] = tanh( sum_k w[n,k,c] * z[b, idx[n,k], t, c] )
  w[n,k,c] = adjacency[n,k] * sum_bb channel_coeffs[c,bb] * basis_weights[bb,n,k]

Decomposition used on device (per core, nodes sharded 8 ways):
  G[bb,n,f]   = sum_k (adj*basis)[bb,n,k] * z_cat[idx[n,k], f]   (PE, k-contraction)
  out[n,f]    = tanh( sum_bb coeffs[c(f),bb] * G[bb,n,f] )       (DVE mask + PE quad-reduce + ACT)
with f = (b, t, c) fused so each gathered row is 8KB (both batches).
"""

import sys

if "/opt/trn_rl_repo" not in sys.path:
    sys.path.insert(0, "/opt/trn_rl_repo")

import numpy as np

import concourse.bass as bass
import concourse.tile as tile
from concourse import bacc, mybir
from concourse.bass_utils import run_bass_kernel_spmd

# Problem constants (nn_CausalGraphLayer_22050362098277)
B, N, T, C = 2, 2048, 32, 32
NUM_BASES, K_CURR = 4, 16
N_CORES = 8
N_LOC = N // N_CORES            # 256 nodes per core
GROUP = 8                       # nodes per gather tile (8 nodes x 16 neigh = 128 slots)
N_GROUPS = N_LOC // GROUP       # 32 groups per core
STACK = 4                       # groups per 128-partition stack
N_STACKS = N_GROUPS // STACK    # 8 stacks per core
F = B * T * C                   # 2048 fused free dim
FCH = 512                       # PSUM bank chunk (fp32)
N_CH = F // FCH                 # 4 chunks

_compiled_cache = {}


def _build_program(reps: int = 1):
    """Build the SPMD Bass program (identical on all cores; per-core data
    differences are carried in the input tensors)."""
    nc = bacc.Bacc(
        "TRN2", target_bir_lowering=False, debug=False, num_devices=N_CORES
    )
    f32 = mybir.dt.float32
    z_cat = nc.dram_tensor("z_cat", [N, F], f32, kind="ExternalInput").ap()
    idx_d = nc.dram_tensor("idx", [128, N_GROUPS], mybir.dt.int32,
                           kind="ExternalInput").ap()
    w_d = nc.dram_tensor("w", [128, N_GROUPS * 32], f32, kind="ExternalInput").ap()
    cm_d = nc.dram_tensor("cm", [128, F], f32, kind="ExternalInput").ap()
    sel_d = nc.dram_tensor("sel", [128, 32], f32, kind="ExternalInput").ap()
    out_d = nc.dram_tensor("out", [N_LOC, F], f32, kind="ExternalOutput").ap()

    with tile.TileContext(nc) as tc:
        with (
            tc.tile_pool(name="const", bufs=1) as cpool,
            tc.tile_pool(name="rbuf", bufs=8) as rpool,
            tc.tile_pool(name="tmp", bufs=4) as tpool,
            tc.tile_pool(name="obuf", bufs=2) as opool,
            tc.tile_pool(name="gps", bufs=4, space="PSUM") as gpool,
            tc.tile_pool(name="ops", bufs=2, space="PSUM") as o3pool,
        ):
            cm_sb = cpool.tile([128, F], f32)
            nc.sync.dma_start(cm_sb[:], cm_d[:, :])
            sel_sb = cpool.tile([128, 32], f32)
            nc.sync.dma_start(sel_sb[:], sel_d[:, :])
            w_sb = cpool.tile([128, N_GROUPS * 32], f32)
            nc.sync.dma_start(w_sb[:], w_d[:, :])
            idx_sb = cpool.tile([128, N_GROUPS], mybir.dt.int32)
            nc.sync.dma_start(idx_sb[:], idx_d[:, :])

            for _rep in range(reps):
                for blk in range(N_STACKS // STACK):       # 2 blocks of 128 nodes
                    outbuf = opool.tile([128, F], f32)
                    for sq in range(STACK):                # 4 stacks of 32 nodes
                        s = STACK * blk + sq
                        rts = []
                        for gq in range(STACK):
                            g = STACK * s + gq
                            r = rpool.tile([128, F], f32)
                            nc.gpsimd.indirect_dma_start(
                                out=r[:],
                                out_offset=None,
                                in_=z_cat[:],
                                in_offset=bass.IndirectOffsetOnAxis(
                                    ap=idx_sb[:, g:g + 1], axis=0
                                ),
                            )
                            rts.append(r)
                        for ch in range(N_CH):
                            gc = gpool.tile([128, FCH], f32)
                            for gq in range(STACK):
                                g = STACK * s + gq
                                nc.tensor.matmul(
                                    out=gc[32 * gq:32 * (gq + 1), :],
                                    lhsT=w_sb[:, 32 * g:32 * (g + 1)],
                                    rhs=rts[gq][:, FCH * ch:FCH * (ch + 1)],
                                    start=True, stop=True,
                                    tile_position=(0, 32 * gq),
                                )
                            tmpc = tpool.tile([128, FCH], f32)
                            nc.vector.tensor_tensor(
                                out=tmpc[:], in0=gc[:],
                                in1=cm_sb[:, FCH * ch:FCH * (ch + 1)],
                                op=mybir.AluOpType.mult,
                            )
                            o3 = o3pool.tile([32, FCH], f32)
                            nc.tensor.matmul(
                                out=o3[:], lhsT=sel_sb[:], rhs=tmpc[:],
                                start=True, stop=True, tile_position=(0, 0),
                            )
                            nc.scalar.activation(
                                out=outbuf[32 * sq:32 * (sq + 1),
                                           FCH * ch:FCH * (ch + 1)],
                                in_=o3[:],
                                func=mybir.ActivationFunctionType.Tanh,
                            )
                    nc.sync.dma_start(
                        out_d[128 * blk:128 * (blk + 1), :], outbuf[:]
                    )

    nc.compile()
    return nc


def _prep_inputs(z, neighbor_indices, adjacency, basis_weights, channel_coeffs):
    """Host-side packing of inputs into the per-core device tensors."""
    z = np.asarray(z, dtype=np.float32)
    nbr = np.asarray(neighbor_indices).astype(np.int32)      # [N, 16]
    adj = np.asarray(adjacency, dtype=np.float32)[:, :K_CURR]
    basis = np.asarray(basis_weights, dtype=np.float32)[:, :, :K_CURR]
    coeffs = np.asarray(channel_coeffs, dtype=np.float32)    # [C, NUM_BASES]

    # z_cat[m, b*T*C + t*C + c] = z[b, m, t, c]
    z_cat = np.ascontiguousarray(
        z.transpose(1, 0, 2, 3).reshape(N, F)
    )

    # wb[bb, n, k] = adj[n,k] * basis[bb,n,k]
    wb = adj[None, :, :] * basis                              # [4, N, 16]

    # coeff mask CM[p, f] = coeffs[c(f), p % 4]
    c_of_f = np.tile(np.arange(C), B * T)                     # [F]
    CM = np.ascontiguousarray(
        coeffs[c_of_f[None, :], (np.arange(128) % NUM_BASES)[:, None]]
    ).astype(np.float32)                                      # [128, F]

    # SEL[p, m] = 1 iff p == 32*(m//8) + 4*(m%8) + bb  for some bb
    p = np.arange(128)
    m = np.arange(32)
    SEL = ((p[:, None] // 32 == m[None, :] // 8)
           & ((p[:, None] % 32) // 4 == m[None, :] % 8)).astype(np.float32)

    in_maps = []
    ii = np.arange(GROUP)
    for r in range(N_CORES):
        lo = r * N_LOC
        nbr_c = nbr[lo:lo + N_LOC]                            # [256, 16]
        idx_core = nbr_c.reshape(N_GROUPS, GROUP * K_CURR)    # [32, 128]
        idx_dram = np.ascontiguousarray(idx_core.T)           # [128, 32]

        wb_c = wb[:, lo:lo + N_LOC, :]                        # [4, 256, 16]
        w_g = wb_c.reshape(NUM_BASES, N_GROUPS, GROUP, K_CURR)
        w_g = w_g.transpose(1, 2, 3, 0)                       # [g, i, k, bb]
        W5 = np.zeros((N_GROUPS, GROUP, K_CURR, GROUP, NUM_BASES),
                      dtype=np.float32)
        W5[:, ii, :, ii, :] = w_g.transpose(1, 0, 2, 3)
        w_dram = np.ascontiguousarray(
            W5.reshape(N_GROUPS, 128, 32).transpose(1, 0, 2).reshape(128, -1)
        )

        in_maps.append({
            "z_cat": z_cat,
            "idx": idx_dram,
            "w": w_dram,
            "cm": CM,
            "sel": SEL,
        })
    return in_maps


def kernel(z, neighbor_indices, adjacency, basis_weights, channel_coeffs):
    assert z.shape == (B, N, T, C), z.shape
    key = "prog1"
    if key not in _compiled_cache:
        _compiled_cache[key] = _build_program(reps=1)
    nc = _compiled_cache[key]

    in_maps = _prep_inputs(z, neighbor_indices, adjacency,
                           basis_weights, channel_coeffs)
    res = run_bass_kernel_spmd(nc, in_maps, core_ids=list(range(N_CORES)))
    out_cat = np.stack([r["out"] for r in res.results])       # [8, 256, F]
    out = (out_cat.reshape(N, B, T, C).transpose(1, 0, 2, 3))
    return np.ascontiguousarray(out)



# revision 2
# speedup vs baseline: 5.3628x; 5.3628x over previous
"""Trainium2 Bass kernel for CausalGraphLayer (gnn message passing).

out[b,n,t,c] = tanh( sum_k w[n,k,c] * z[b, idx[n,k], t, c] )
  w[n,k,c] = adjacency[n,k] * sum_bb channel_coeffs[c,bb] * basis_weights[bb,n,k]

Device decomposition (per core, nodes sharded 8 ways):
  G[bb,n,f]   = sum_k (adj*basis)[bb,n,k] * z_cat[idx[n,k], f]   (PE, k-contraction)
  out[n,f]    = tanh( sum_bb coeffs[c(f),bb] * G[bb,n,f] )       (DVE mask + PE quad-reduce + ACT)
with f = (b, t, c) fused so each gathered row is one DMA descriptor.

Performance path (vs the fp32 original):
  - z / w / sel / tmpc / out in bf16 (rel_err ~3e-3, gate is 2e-2):
    halves HBM gather traffic and wire traffic.
  - replicated inputs (z_cat, cm, sel) shipped over the slow axon link
    ONCE as shards and replicated on-device via an XLA all_gather (ICI).
  - the jax.jit(shard_map(...)) dispatcher is built once and cached —
    upstream run_bass_kernel_spmd rebuilds it per call (seconds).
  - donated output zero-buffers are created on device, not shipped.
"""

import sys

if "/opt/trn_rl_repo" not in sys.path:
    sys.path.insert(0, "/opt/trn_rl_repo")

import numpy as np
import ml_dtypes

import concourse.bass as bass
import concourse.tile as tile
from concourse import bacc, mybir

# Problem constants (nn_CausalGraphLayer_22050362098277)
B, N, T, C = 2, 2048, 32, 32
NUM_BASES, K_CURR = 4, 16
N_CORES = 8
N_LOC = N // N_CORES            # 256 nodes per core
GROUP = 8                       # nodes per gather tile (8 nodes x 16 neigh = 128 slots)
N_GROUPS = N_LOC // GROUP       # 32 groups per core
STACK = 4                       # groups per 128-partition stack
N_STACKS = N_GROUPS // STACK    # 8 stacks per core
F = B * T * C                   # 2048 fused free dim
FCH = 512                       # PSUM bank chunk (fp32)
N_CH = F // FCH                 # 4 chunks

BF16 = ml_dtypes.bfloat16

_compiled_cache = {}

# Inputs identical on every core (replicated on device via all_gather).
_REPL_NAMES = ("z_cat", "cm", "sel")


def _build_program(reps: int = 1):
    nc = bacc.Bacc(
        "TRN2", target_bir_lowering=False, debug=False, num_devices=N_CORES
    )
    f32 = mybir.dt.float32
    bf16 = mybir.dt.bfloat16
    z_cat = nc.dram_tensor("z_cat", [N, F], bf16, kind="ExternalInput").ap()
    idx_d = nc.dram_tensor("idx", [128, N_GROUPS], mybir.dt.int32,
                           kind="ExternalInput").ap()
    w_d = nc.dram_tensor("w", [128, N_GROUPS * 32], bf16, kind="ExternalInput").ap()
    cm_d = nc.dram_tensor("cm", [128, F], f32, kind="ExternalInput").ap()
    sel_d = nc.dram_tensor("sel", [128, 32], bf16, kind="ExternalInput").ap()
    out_d = nc.dram_tensor("out", [N_LOC, F], bf16, kind="ExternalOutput").ap()

    with tile.TileContext(nc) as tc:
        with (
            tc.tile_pool(name="const", bufs=1) as cpool,
            tc.tile_pool(name="rbuf", bufs=12) as rpool,
            tc.tile_pool(name="tmp", bufs=4) as tpool,
            tc.tile_pool(name="obuf", bufs=2) as opool,
            tc.tile_pool(name="gps", bufs=4, space="PSUM") as gpool,
            tc.tile_pool(name="ops", bufs=2, space="PSUM") as o3pool,
        ):
            cm_sb = cpool.tile([128, F], f32)
            nc.sync.dma_start(cm_sb[:], cm_d[:, :])
            sel_sb = cpool.tile([128, 32], bf16)
            nc.sync.dma_start(sel_sb[:], sel_d[:, :])
            w_sb = cpool.tile([128, N_GROUPS * 32], bf16)
            nc.sync.dma_start(w_sb[:], w_d[:, :])
            idx_sb = cpool.tile([128, N_GROUPS], mybir.dt.int32)
            nc.sync.dma_start(idx_sb[:], idx_d[:, :])

            for _rep in range(reps):
                for blk in range(N_STACKS // STACK):       # 2 blocks of 128 nodes
                    outbuf = opool.tile([128, F], bf16)
                    for sq in range(STACK):                # 4 stacks of 32 nodes
                        s = STACK * blk + sq
                        rts = []
                        for gq in range(STACK):
                            g = STACK * s + gq
                            r = rpool.tile([128, F], bf16)
                            nc.gpsimd.indirect_dma_start(
                                out=r[:],
                                out_offset=None,
                                in_=z_cat[:],
                                in_offset=bass.IndirectOffsetOnAxis(
                                    ap=idx_sb[:, g:g + 1], axis=0
                                ),
                            )
                            rts.append(r)
                        for ch in range(N_CH):
                            gc = gpool.tile([128, FCH], f32)
                            for gq in range(STACK):
                                g = STACK * s + gq
                                nc.tensor.matmul(
                                    out=gc[32 * gq:32 * (gq + 1), :],
                                    lhsT=w_sb[:, 32 * g:32 * (g + 1)],
                                    rhs=rts[gq][:, FCH * ch:FCH * (ch + 1)],
                                    start=True, stop=True,
                                    tile_position=(0, 32 * gq),
                                )
                            tmpc = tpool.tile([128, FCH], bf16)
                            nc.vector.tensor_tensor(
                                out=tmpc[:], in0=gc[:],
                                in1=cm_sb[:, FCH * ch:FCH * (ch + 1)],
                                op=mybir.AluOpType.mult,
                            )
                            o3 = o3pool.tile([32, FCH], f32)
                            nc.tensor.matmul(
                                out=o3[:], lhsT=sel_sb[:], rhs=tmpc[:],
                                start=True, stop=True, tile_position=(0, 0),
                            )
                            nc.scalar.activation(
                                out=outbuf[32 * sq:32 * (sq + 1),
                                           FCH * ch:FCH * (ch + 1)],
                                in_=o3[:],
                                func=mybir.ActivationFunctionType.Tanh,
                            )
                    nc.sync.dma_start(
                        out_d[128 * blk:128 * (blk + 1), :], outbuf[:]
                    )

    nc.compile()
    return nc


def _prep_common(z, neighbor_indices, adjacency, basis_weights, channel_coeffs):
    """Host-side packing. Returns (replicated_map, per_core_concat_map)."""
    z = np.asarray(z, dtype=np.float32)
    nbr = np.asarray(neighbor_indices).astype(np.int32)      # [N, 16]
    adj = np.asarray(adjacency, dtype=np.float32)[:, :K_CURR]
    basis = np.asarray(basis_weights, dtype=np.float32)[:, :, :K_CURR]
    coeffs = np.asarray(channel_coeffs, dtype=np.float32)    # [C, NUM_BASES]

    # z_cat[m, b*T*C + t*C + c] = z[b, m, t, c]
    z_cat = np.ascontiguousarray(
        z.transpose(1, 0, 2, 3).reshape(N, F)
    ).astype(BF16)

    # wb[bb, n, k] = adj[n,k] * basis[bb,n,k]
    wb = adj[None, :, :] * basis                              # [4, N, 16]

    # coeff mask CM[p, f] = coeffs[c(f), p % 4]
    c_of_f = np.tile(np.arange(C), B * T)                     # [F]
    CM = np.ascontiguousarray(
        coeffs[c_of_f[None, :], (np.arange(128) % NUM_BASES)[:, None]]
    ).astype(np.float32)                                      # [128, F]

    # SEL[p, m] = 1 iff p == 32*(m//8) + 4*(m%8) + bb  for some bb
    p = np.arange(128)
    m = np.arange(32)
    SEL = ((p[:, None] // 32 == m[None, :] // 8)
           & ((p[:, None] % 32) // 4 == m[None, :] % 8)).astype(BF16)

    idx_list, w_list = [], []
    ii = np.arange(GROUP)
    for r in range(N_CORES):
        lo = r * N_LOC
        nbr_c = nbr[lo:lo + N_LOC]                            # [256, 16]
        idx_core = nbr_c.reshape(N_GROUPS, GROUP * K_CURR)    # [32, 128]
        idx_list.append(np.ascontiguousarray(idx_core.T))     # [128, 32]

        wb_c = wb[:, lo:lo + N_LOC, :]                        # [4, 256, 16]
        w_g = wb_c.reshape(NUM_BASES, N_GROUPS, GROUP, K_CURR)
        w_g = w_g.transpose(1, 2, 3, 0)                       # [g, i, k, bb]
        W5 = np.zeros((N_GROUPS, GROUP, K_CURR, GROUP, NUM_BASES),
                      dtype=np.float32)
        W5[:, ii, :, ii, :] = w_g.transpose(1, 0, 2, 3)
        w_list.append(np.ascontiguousarray(
            W5.reshape(N_GROUPS, 128, 32).transpose(1, 0, 2).reshape(128, -1)
        ).astype(BF16))

    repl = {"z_cat": z_cat, "cm": CM, "sel": SEL}
    per_core = {
        "idx": np.concatenate(idx_list, axis=0),
        "w": np.concatenate(w_list, axis=0),
    }
    return repl, per_core


def _make_runner(nc, n_cores):
    """Cached jitted SPMD dispatcher (see module docstring)."""
    import jax
    import jax.numpy as jnp
    from jax.sharding import Mesh, PartitionSpec as P, NamedSharding
    from jax.experimental.shard_map import shard_map
    from concourse.bass2jax import (
        _bass_exec_p,
        install_neuronx_cc_hook,
        partition_id_tensor,
    )

    install_neuronx_cc_hook()
    assert nc.dbg_addr is None or not nc.dbg_callbacks

    partition_name = (
        nc.partition_id_tensor.name if nc.partition_id_tensor else None
    )
    in_names, out_names, out_avals, zero_shapes = [], [], [], []
    for alloc in nc.m.functions[0].allocations:
        if not isinstance(alloc, mybir.MemoryLocationSet):
            continue
        name = alloc.memorylocations[0].name
        if alloc.kind == "ExternalInput":
            if name != partition_name:
                in_names.append(name)
        elif alloc.kind == "ExternalOutput":
            out_names.append(name)
            shape = tuple(alloc.tensor_shape)
            dtype = mybir.dt.np(alloc.dtype)
            out_avals.append(jax.core.ShapedArray(shape, dtype))
            zero_shapes.append((shape, dtype))
    n_params = len(in_names)
    n_outs = len(out_avals)
    all_in_names = list(in_names) + list(out_names)
    if partition_name is not None:
        all_in_names.append(partition_name)
    donate = tuple(range(n_params, n_params + n_outs))

    def _body(*args):
        operands = list(args)
        if partition_name is not None:
            operands.append(partition_id_tensor())
        outs = _bass_exec_p.bind(
            *operands,
            out_avals=tuple(out_avals),
            in_names=tuple(all_in_names),
            out_names=tuple(out_names),
            lowering_input_output_aliases=(),
            sim_require_finite=True,
            sim_require_nnan=True,
            nc=nc,
        )
        return tuple(outs)

    devices = jax.devices()[:n_cores]
    assert len(devices) == n_cores
    mesh = Mesh(np.asarray(devices), ("core",))

    in_specs = tuple(
        P() if name in _REPL_NAMES else P("core") for name in in_names
    ) + (P("core"),) * n_outs
    out_specs = (P("core"),) * n_outs
    sharded = jax.jit(
        shard_map(_body, mesh=mesh, in_specs=in_specs,
                  out_specs=out_specs, check_rep=False),
        donate_argnums=donate,
        keep_unused=True,
    )

    repl_names = [n for n in in_names if n in _REPL_NAMES]

    def _gather_body(*xs):
        return tuple(
            jax.lax.all_gather(x, "core", axis=0, tiled=True) for x in xs
        )

    gatherer = jax.jit(
        shard_map(_gather_body, mesh=mesh,
                  in_specs=(P("core"),) * len(repl_names),
                  out_specs=(P(),) * len(repl_names),
                  check_rep=False)
    )

    zeros_fns = [
        jax.jit(
            lambda s=s, d=d: jnp.zeros((n_cores * s[0], *s[1:]), d),
            out_shardings=NamedSharding(mesh, P("core")),
        )
        for (s, d) in zero_shapes
    ]

    def run(repl, per_core):
        repl_dev = dict(zip(repl_names, gatherer(*[repl[n] for n in repl_names])))
        args = [
            repl_dev[n] if n in repl_dev else per_core[n] for n in in_names
        ]
        zeros = [zf() for zf in zeros_fns]
        out_arrs = sharded(*args, *zeros)
        outs = {}
        for i, name in enumerate(out_names):
            s, d = zero_shapes[i]
            outs[name] = np.asarray(out_arrs[i]).reshape(n_cores, *s)
        return outs

    return run


def kernel(z, neighbor_indices, adjacency, basis_weights, channel_coeffs):
    assert z.shape == (B, N, T, C), z.shape
    key = "prog"
    if key not in _compiled_cache:
        nc = _build_program(reps=1)
        _compiled_cache[key] = (nc, _make_runner(nc, N_CORES))
    nc, run = _compiled_cache[key]

    repl, per_core = _prep_common(z, neighbor_indices, adjacency,
                                  basis_weights, channel_coeffs)
    out_cat = run(repl, per_core)["out"]                      # [8, 256, F] bf16
    out = (out_cat.astype(np.float32)
           .reshape(N, B, T, C).transpose(1, 0, 2, 3))
    return np.ascontiguousarray(out)


# revision 8
# speedup vs baseline: 5.3755x; 1.0024x over previous
"""Trainium2 Bass kernel for CausalGraphLayer (gnn message passing).

out[b,n,t,c] = tanh( sum_k w[n,k,c] * z[b, idx[n,k], t, c] )
  w[n,k,c] = adjacency[n,k] * sum_bb channel_coeffs[c,bb] * basis_weights[bb,n,k]

Device decomposition (per core, nodes sharded 8 ways):
  G[bb,n,f]   = sum_k (adj*basis)[bb,n,k] * z_cat[idx[n,k], f]   (PE, k-contraction)
  out[n,f]    = tanh( sum_bb coeffs[c(f),bb] * G[bb,n,f] )       (DVE mask + PE quad-reduce + ACT)
with f = (b, t, c) fused so each gathered row is one DMA descriptor.

Performance path (vs the fp32 original):
  - z / w / sel / tmpc / out in bf16 (rel_err ~3e-3, gate is 2e-2):
    halves HBM gather traffic and wire traffic.
  - gather via gpsimd.dma_gather, 512 rows per instruction on 4 SWDGE
    queues (vs 4x indirect_dma_start per stack): ~18% less device time
    (SWDGE descriptor emission is the device bottleneck at ~120 ns/row).
  - replicated inputs (z_cat, cm, sel) shipped over the slow axon link
    ONCE as shards and replicated on-device via an XLA all_gather (ICI).
  - the jax.jit(shard_map(...)) dispatcher is built once and cached —
    upstream run_bass_kernel_spmd rebuilds it per call (seconds).
  - donated output zero-buffers are created on device, not shipped.
"""

import sys

if "/opt/trn_rl_repo" not in sys.path:
    sys.path.insert(0, "/opt/trn_rl_repo")

import numpy as np
import ml_dtypes

import concourse.bass as bass
import concourse.tile as tile
from concourse import bacc, mybir

# Problem constants (nn_CausalGraphLayer_22050362098277)
B, N, T, C = 2, 2048, 32, 32
NUM_BASES, K_CURR = 4, 16
N_CORES = 8
N_LOC = N // N_CORES            # 256 nodes per core
GROUP = 8                       # nodes per gather tile (8 nodes x 16 neigh = 128 slots)
N_GROUPS = N_LOC // GROUP       # 32 groups per core
STACK = 4                       # groups per 128-partition stack
N_STACKS = N_GROUPS // STACK    # 8 stacks per core
F = B * T * C                   # 2048 fused free dim
FCH = 512                       # PSUM bank chunk (fp32)
N_CH = F // FCH                 # 4 chunks
IDXC = 512 // 16                # dma_gather idx cols per stack (16-row wrap)

BF16 = ml_dtypes.bfloat16

_compiled_cache = {}

# Inputs identical on every core (replicated on device via all_gather).
_REPL_NAMES = ("z_cat", "cm", "sel")


def _build_program(reps: int = 1):
    nc = bacc.Bacc(
        "TRN2", target_bir_lowering=False, debug=False, num_devices=N_CORES,
        num_swdge_queues=4,
    )
    f32 = mybir.dt.float32
    bf16 = mybir.dt.bfloat16
    z_cat = nc.dram_tensor("z_cat", [N, F], bf16, kind="ExternalInput").ap()
    idx_d = nc.dram_tensor("idx", [128, N_STACKS * IDXC], mybir.dt.int16,
                           kind="ExternalInput").ap()
    w_d = nc.dram_tensor("w", [128, N_GROUPS * 32], bf16, kind="ExternalInput").ap()
    cm_d = nc.dram_tensor("cm", [128, F], f32, kind="ExternalInput").ap()
    sel_d = nc.dram_tensor("sel", [128, 32], bf16, kind="ExternalInput").ap()
    out_d = nc.dram_tensor("out", [N_LOC, F], bf16, kind="ExternalOutput").ap()

    with tile.TileContext(nc) as tc:
        with (
            tc.tile_pool(name="const", bufs=1) as cpool,
            tc.tile_pool(name="rbuf", bufs=4) as rpool,
            tc.tile_pool(name="tmp", bufs=4) as tpool,
            tc.tile_pool(name="obuf", bufs=2) as opool,
            tc.tile_pool(name="gps", bufs=4, space="PSUM") as gpool,
            tc.tile_pool(name="ops", bufs=2, space="PSUM") as o3pool,
        ):
            cm_sb = cpool.tile([128, F], f32)
            nc.sync.dma_start(cm_sb[:], cm_d[:, :])
            sel_sb = cpool.tile([128, 32], bf16)
            nc.sync.dma_start(sel_sb[:], sel_d[:, :])
            w_sb = cpool.tile([128, N_GROUPS * 32], bf16)
            nc.sync.dma_start(w_sb[:], w_d[:, :])
            idx_sb = cpool.tile([128, N_STACKS * IDXC], mybir.dt.int16)
            nc.sync.dma_start(idx_sb[:], idx_d[:, :])

            for _rep in range(reps):
                for blk in range(N_STACKS // STACK):       # 2 blocks of 128 nodes
                    outbuf = opool.tile([128, F], bf16)
                    for sq in range(STACK):                # 4 stacks of 32 nodes
                        s = STACK * blk + sq
                        r3 = rpool.tile([128, STACK, F], bf16)
                        nc.gpsimd.dma_gather(
                            r3[:, :, :],
                            z_cat[:, :],
                            idx_sb[:, IDXC * s:IDXC * (s + 1)],
                            STACK * 128,          # num_idxs
                            STACK * 128,          # num_idxs_reg
                            F,                    # elem_size
                            queue_num=s % 4,
                        )
                        for ch in range(N_CH):
                            gc = gpool.tile([128, FCH], f32)
                            for gq in range(STACK):
                                g = STACK * s + gq
                                nc.tensor.matmul(
                                    out=gc[32 * gq:32 * (gq + 1), :],
                                    lhsT=w_sb[:, 32 * g:32 * (g + 1)],
                                    rhs=r3[:, gq, FCH * ch:FCH * (ch + 1)],
                                    start=True, stop=True,
                                    tile_position=(0, 32 * gq),
                                )
                            tmpc = tpool.tile([128, FCH], bf16)
                            nc.vector.tensor_tensor(
                                out=tmpc[:], in0=gc[:],
                                in1=cm_sb[:, FCH * ch:FCH * (ch + 1)],
                                op=mybir.AluOpType.mult,
                            )
                            o3 = o3pool.tile([32, FCH], f32)
                            nc.tensor.matmul(
                                out=o3[:], lhsT=sel_sb[:], rhs=tmpc[:],
                                start=True, stop=True, tile_position=(0, 0),
                            )
                            nc.scalar.activation(
                                out=outbuf[32 * sq:32 * (sq + 1),
                                           FCH * ch:FCH * (ch + 1)],
                                in_=o3[:],
                                func=mybir.ActivationFunctionType.Tanh,
                            )
                    nc.sync.dma_start(
                        out_d[128 * blk:128 * (blk + 1), :], outbuf[:]
                    )

    nc.compile()
    return nc


def _prep_common(z, neighbor_indices, adjacency, basis_weights, channel_coeffs):
    """Host-side packing. Returns (replicated_map, per_core_concat_map)."""
    z = np.asarray(z, dtype=np.float32)
    nbr = np.asarray(neighbor_indices).astype(np.int32)      # [N, 16]
    adj = np.asarray(adjacency, dtype=np.float32)[:, :K_CURR]
    basis = np.asarray(basis_weights, dtype=np.float32)[:, :, :K_CURR]
    coeffs = np.asarray(channel_coeffs, dtype=np.float32)    # [C, NUM_BASES]

    # z_cat[m, b*T*C + t*C + c] = z[b, m, t, c]
    z_cat = np.ascontiguousarray(
        z.transpose(1, 0, 2, 3).reshape(N, F)
    ).astype(BF16)

    # wb[bb, n, k] = adj[n,k] * basis[bb,n,k]
    wb = adj[None, :, :] * basis                              # [4, N, 16]

    # coeff mask CM[p, f] = coeffs[c(f), p % 4]
    c_of_f = np.tile(np.arange(C), B * T)                     # [F]
    CM = np.ascontiguousarray(
        coeffs[c_of_f[None, :], (np.arange(128) % NUM_BASES)[:, None]]
    ).astype(np.float32)                                      # [128, F]

    # SEL[p, m] = 1 iff p == 32*(m//8) + 4*(m%8) + bb  for some bb
    p = np.arange(128)
    m = np.arange(32)
    SEL = ((p[:, None] // 32 == m[None, :] // 8)
           & ((p[:, None] % 32) // 4 == m[None, :] % 8)).astype(BF16)

    idx_list, w_list = [], []
    ii = np.arange(GROUP)
    for r in range(N_CORES):
        lo = r * N_LOC
        nbr_c = nbr[lo:lo + N_LOC]                            # [256, 16]
        idx_core = nbr_c.reshape(N_GROUPS, GROUP * K_CURR)    # [32 groups, 128]
        # dma_gather index layout per stack: slot i = j*128 + p is read
        # from partition i%16, col i//16; replicate [16, IDXC] to 128 rows.
        cols = []
        for s in range(N_STACKS):
            ids = idx_core[STACK * s:STACK * (s + 1)].reshape(-1)  # [512]
            wrapped = ids.reshape(IDXC, 16).T                      # [16, IDXC]
            cols.append(np.tile(wrapped, (8, 1)))                  # [128, IDXC]
        idx_list.append(np.ascontiguousarray(
            np.concatenate(cols, axis=1).astype(np.int16)))

        wb_c = wb[:, lo:lo + N_LOC, :]                        # [4, 256, 16]
        w_g = wb_c.reshape(NUM_BASES, N_GROUPS, GROUP, K_CURR)
        w_g = w_g.transpose(1, 2, 3, 0)                       # [g, i, k, bb]
        W5 = np.zeros((N_GROUPS, GROUP, K_CURR, GROUP, NUM_BASES),
                      dtype=np.float32)
        W5[:, ii, :, ii, :] = w_g.transpose(1, 0, 2, 3)
        w_list.append(np.ascontiguousarray(
            W5.reshape(N_GROUPS, 128, 32).transpose(1, 0, 2).reshape(128, -1)
        ).astype(BF16))

    repl = {"z_cat": z_cat, "cm": CM, "sel": SEL}
    per_core = {
        "idx": np.concatenate(idx_list, axis=0),
        "w": np.concatenate(w_list, axis=0),
    }
    return repl, per_core


def _make_runner(nc, n_cores):
    """Cached jitted SPMD dispatcher (see module docstring)."""
    import jax
    import jax.numpy as jnp
    from jax.sharding import Mesh, PartitionSpec as P, NamedSharding
    from jax.experimental.shard_map import shard_map
    from concourse.bass2jax import (
        _bass_exec_p,
        install_neuronx_cc_hook,
        partition_id_tensor,
    )

    install_neuronx_cc_hook()
    assert nc.dbg_addr is None or not nc.dbg_callbacks

    partition_name = (
        nc.partition_id_tensor.name if nc.partition_id_tensor else None
    )
    in_names, out_names, out_avals, zero_shapes = [], [], [], []
    for alloc in nc.m.functions[0].allocations:
        if not isinstance(alloc, mybir.MemoryLocationSet):
            continue
        name = alloc.memorylocations[0].name
        if alloc.kind == "ExternalInput":
            if name != partition_name:
                in_names.append(name)
        elif alloc.kind == "ExternalOutput":
            out_names.append(name)
            shape = tuple(alloc.tensor_shape)
            dtype = mybir.dt.np(alloc.dtype)
            out_avals.append(jax.core.ShapedArray(shape, dtype))
            zero_shapes.append((shape, dtype))
    n_params = len(in_names)
    n_outs = len(out_avals)
    all_in_names = list(in_names) + list(out_names)
    if partition_name is not None:
        all_in_names.append(partition_name)
    donate = tuple(range(n_params, n_params + n_outs))

    def _body(*args):
        operands = list(args)
        if partition_name is not None:
            operands.append(partition_id_tensor())
        outs = _bass_exec_p.bind(
            *operands,
            out_avals=tuple(out_avals),
            in_names=tuple(all_in_names),
            out_names=tuple(out_names),
            lowering_input_output_aliases=(),
            sim_require_finite=True,
            sim_require_nnan=True,
            nc=nc,
        )
        return tuple(outs)

    devices = jax.devices()[:n_cores]
    assert len(devices) == n_cores
    mesh = Mesh(np.asarray(devices), ("core",))

    in_specs = tuple(
        P() if name in _REPL_NAMES else P("core") for name in in_names
    ) + (P("core"),) * n_outs
    out_specs = (P("core"),) * n_outs
    sharded = jax.jit(
        shard_map(_body, mesh=mesh, in_specs=in_specs,
                  out_specs=out_specs, check_rep=False),
        donate_argnums=donate,
        keep_unused=True,
    )

    repl_names = [n for n in in_names if n in _REPL_NAMES]

    def _gather_body(*xs):
        return tuple(
            jax.lax.all_gather(x, "core", axis=0, tiled=True) for x in xs
        )

    gatherer = jax.jit(
        shard_map(_gather_body, mesh=mesh,
                  in_specs=(P("core"),) * len(repl_names),
                  out_specs=(P(),) * len(repl_names),
                  check_rep=False)
    )

    zeros_fns = [
        jax.jit(
            lambda s=s, d=d: jnp.zeros((n_cores * s[0], *s[1:]), d),
            out_shardings=NamedSharding(mesh, P("core")),
        )
        for (s, d) in zero_shapes
    ]

    def run(repl, per_core):
        repl_dev = dict(zip(repl_names, gatherer(*[repl[n] for n in repl_names])))
        args = [
            repl_dev[n] if n in repl_dev else per_core[n] for n in in_names
        ]
        zeros = [zf() for zf in zeros_fns]
        out_arrs = sharded(*args, *zeros)
        outs = {}
        for i, name in enumerate(out_names):
            s, d = zero_shapes[i]
            outs[name] = np.asarray(out_arrs[i]).reshape(n_cores, *s)
        return outs

    return run


def kernel(z, neighbor_indices, adjacency, basis_weights, channel_coeffs):
    assert z.shape == (B, N, T, C), z.shape
    key = "prog"
    if key not in _compiled_cache:
        nc = _build_program(reps=1)
        _compiled_cache[key] = (nc, _make_runner(nc, N_CORES))
    nc, run = _compiled_cache[key]

    repl, per_core = _prep_common(z, neighbor_indices, adjacency,
                                  basis_weights, channel_coeffs)
    out_cat = run(repl, per_core)["out"]                      # [8, 256, F] bf16
    out = (out_cat.astype(np.float32)
           .reshape(N, B, T, C).transpose(1, 0, 2, 3))
    return np.ascontiguousarray(out)


# revision 11
# speedup vs baseline: 5.8230x; 1.0832x over previous
"""Trainium2 Bass kernel for CausalGraphLayer (gnn message passing).

out[b,n,t,c] = tanh( sum_k w[n,k,c] * z[b, idx[n,k], t, c] )
  w[n,k,c] = adjacency[n,k] * sum_bb channel_coeffs[c,bb] * basis_weights[bb,n,k]

Device decomposition (per core, nodes sharded 8 ways):
  G[bb,n,f]   = sum_k (adj*basis)[bb,n,k] * z_cat[idx[n,k], f]   (PE, k-contraction)
  out[n,f]    = tanh( sum_bb coeffs[c(f),bb] * G[bb,n,f] )       (DVE mask + PE quad-reduce + ACT)
with f = (b, t, c) fused so each gathered row is one DMA descriptor.

Performance path (vs the fp32 original):
  - z / w / sel / tmpc / out in bf16 (rel_err ~3e-3, gate is 2e-2):
    halves HBM gather traffic and wire traffic.
  - gather via gpsimd.dma_gather, 1024 rows per instruction on 4 SWDGE
    queues (vs 4x indirect_dma_start of 128 rows per stack): device time
    605 -> 217 us/iter. SWDGE per-instruction descriptor-emission
    overhead on the Q7 is the device bottleneck, so fewer/bigger gather
    instructions win; HBM roofline for the 16 MB gather is ~45 us.
  - replicated inputs (z_cat, cm, sel) shipped over the slow axon link
    ONCE as shards and replicated on-device via an XLA all_gather (ICI).
  - the jax.jit(shard_map(...)) dispatcher is built once and cached —
    upstream run_bass_kernel_spmd rebuilds it per call (seconds).
  - donated output zero-buffers are created on device, not shipped.
"""

import sys

if "/opt/trn_rl_repo" not in sys.path:
    sys.path.insert(0, "/opt/trn_rl_repo")

import numpy as np
import ml_dtypes

import concourse.bass as bass
import concourse.tile as tile
from concourse import bacc, mybir

# Problem constants (nn_CausalGraphLayer_22050362098277)
B, N, T, C = 2, 2048, 32, 32
NUM_BASES, K_CURR = 4, 16
N_CORES = 8
N_LOC = N // N_CORES            # 256 nodes per core
GROUP = 8                       # nodes per gather tile (8 nodes x 16 neigh = 128 slots)
N_GROUPS = N_LOC // GROUP       # 32 groups per core
STACK = 4                       # groups per 128-partition stack
N_STACKS = N_GROUPS // STACK    # 8 stacks per core
F = B * T * C                   # 2048 fused free dim
FCH = 512                       # PSUM bank chunk (fp32)
N_CH = F // FCH                 # 4 chunks
IDXC = 512 // 16                # dma_gather idx cols per stack (16-row wrap)

BF16 = ml_dtypes.bfloat16

_compiled_cache = {}

# Inputs identical on every core (replicated on device via all_gather).
_REPL_NAMES = ("z_cat", "cm", "sel")


def _build_program(reps: int = 1):
    nc = bacc.Bacc(
        "TRN2", target_bir_lowering=False, debug=False, num_devices=N_CORES,
        num_swdge_queues=4,
    )
    f32 = mybir.dt.float32
    bf16 = mybir.dt.bfloat16
    z_cat = nc.dram_tensor("z_cat", [N, F], bf16, kind="ExternalInput").ap()
    idx_d = nc.dram_tensor("idx", [128, N_STACKS * IDXC], mybir.dt.int16,
                           kind="ExternalInput").ap()
    w_d = nc.dram_tensor("w", [128, N_GROUPS * 32], bf16, kind="ExternalInput").ap()
    cm_d = nc.dram_tensor("cm", [128, F], f32, kind="ExternalInput").ap()
    sel_d = nc.dram_tensor("sel", [128, 32], bf16, kind="ExternalInput").ap()
    out_d = nc.dram_tensor("out", [N_LOC, F], bf16, kind="ExternalOutput").ap()

    with tile.TileContext(nc) as tc:
        with (
            tc.tile_pool(name="const", bufs=1) as cpool,
            tc.tile_pool(name="rbuf", bufs=3) as rpool,
            tc.tile_pool(name="tmp", bufs=4) as tpool,
            tc.tile_pool(name="obuf", bufs=2) as opool,
            tc.tile_pool(name="gps", bufs=4, space="PSUM") as gpool,
            tc.tile_pool(name="ops", bufs=2, space="PSUM") as o3pool,
        ):
            cm_sb = cpool.tile([128, F], f32)
            nc.sync.dma_start(cm_sb[:], cm_d[:, :])
            sel_sb = cpool.tile([128, 32], bf16)
            nc.sync.dma_start(sel_sb[:], sel_d[:, :])
            w_sb = cpool.tile([128, N_GROUPS * 32], bf16)
            nc.sync.dma_start(w_sb[:], w_d[:, :])
            idx_sb = cpool.tile([128, N_STACKS * IDXC], mybir.dt.int16)
            nc.sync.dma_start(idx_sb[:], idx_d[:, :])

            for _rep in range(reps):
                for blk in range(N_STACKS // STACK):       # 2 blocks of 128 nodes
                    outbuf = opool.tile([128, F], bf16)
                    for half in range(2):                  # 2 double-stacks/blk
                        d = 2 * blk + half
                        r3 = rpool.tile([128, 2 * STACK, F], bf16)
                        nc.gpsimd.dma_gather(
                            r3[:, :, :],
                            z_cat[:, :],
                            idx_sb[:, 2 * IDXC * d:2 * IDXC * (d + 1)],
                            1024,                 # num_idxs (8 groups)
                            1024,                 # num_idxs_reg
                            F,                    # elem_size
                            queue_num=d % 4,
                        )
                        for hq in range(2):                # stacks in dbl-stack
                            sq = 2 * half + hq
                            s = STACK * blk + sq
                            for ch in range(N_CH):
                                gc = gpool.tile([128, FCH], f32)
                                for gq in range(STACK):
                                    g = STACK * s + gq
                                    nc.tensor.matmul(
                                        out=gc[32 * gq:32 * (gq + 1), :],
                                        lhsT=w_sb[:, 32 * g:32 * (g + 1)],
                                        rhs=r3[:, 4 * hq + gq,
                                               FCH * ch:FCH * (ch + 1)],
                                        start=True, stop=True,
                                        tile_position=(0, 32 * gq),
                                    )
                                tmpc = tpool.tile([128, FCH], bf16)
                                nc.vector.tensor_tensor(
                                    out=tmpc[:], in0=gc[:],
                                    in1=cm_sb[:, FCH * ch:FCH * (ch + 1)],
                                    op=mybir.AluOpType.mult,
                                )
                                o3 = o3pool.tile([32, FCH], f32)
                                nc.tensor.matmul(
                                    out=o3[:], lhsT=sel_sb[:], rhs=tmpc[:],
                                    start=True, stop=True, tile_position=(0, 0),
                                )
                                nc.scalar.activation(
                                    out=outbuf[32 * sq:32 * (sq + 1),
                                               FCH * ch:FCH * (ch + 1)],
                                    in_=o3[:],
                                    func=mybir.ActivationFunctionType.Tanh,
                                )
                    nc.sync.dma_start(
                        out_d[128 * blk:128 * (blk + 1), :], outbuf[:]
                    )

    nc.compile()
    return nc


def _prep_common(z, neighbor_indices, adjacency, basis_weights, channel_coeffs):
    """Host-side packing. Returns (replicated_map, per_core_concat_map)."""
    z = np.asarray(z, dtype=np.float32)
    nbr = np.asarray(neighbor_indices).astype(np.int32)      # [N, 16]
    adj = np.asarray(adjacency, dtype=np.float32)[:, :K_CURR]
    basis = np.asarray(basis_weights, dtype=np.float32)[:, :, :K_CURR]
    coeffs = np.asarray(channel_coeffs, dtype=np.float32)    # [C, NUM_BASES]

    # z_cat[m, b*T*C + t*C + c] = z[b, m, t, c]
    z_cat = np.ascontiguousarray(
        z.transpose(1, 0, 2, 3).reshape(N, F)
    ).astype(BF16)

    # wb[bb, n, k] = adj[n,k] * basis[bb,n,k]
    wb = adj[None, :, :] * basis                              # [4, N, 16]

    # coeff mask CM[p, f] = coeffs[c(f), p % 4]
    c_of_f = np.tile(np.arange(C), B * T)                     # [F]
    CM = np.ascontiguousarray(
        coeffs[c_of_f[None, :], (np.arange(128) % NUM_BASES)[:, None]]
    ).astype(np.float32)                                      # [128, F]

    # SEL[p, m] = 1 iff p == 32*(m//8) + 4*(m%8) + bb  for some bb
    p = np.arange(128)
    m = np.arange(32)
    SEL = ((p[:, None] // 32 == m[None, :] // 8)
           & ((p[:, None] % 32) // 4 == m[None, :] % 8)).astype(BF16)

    idx_list, w_list = [], []
    ii = np.arange(GROUP)
    for r in range(N_CORES):
        lo = r * N_LOC
        nbr_c = nbr[lo:lo + N_LOC]                            # [256, 16]
        idx_core = nbr_c.reshape(N_GROUPS, GROUP * K_CURR)    # [32 groups, 128]
        # dma_gather index layout per stack: slot i = j*128 + p is read
        # from partition i%16, col i//16; replicate [16, IDXC] to 128 rows.
        cols = []
        for s in range(N_STACKS):
            ids = idx_core[STACK * s:STACK * (s + 1)].reshape(-1)  # [512]
            wrapped = ids.reshape(IDXC, 16).T                      # [16, IDXC]
            cols.append(np.tile(wrapped, (8, 1)))                  # [128, IDXC]
        idx_list.append(np.ascontiguousarray(
            np.concatenate(cols, axis=1).astype(np.int16)))

        wb_c = wb[:, lo:lo + N_LOC, :]                        # [4, 256, 16]
        w_g = wb_c.reshape(NUM_BASES, N_GROUPS, GROUP, K_CURR)
        w_g = w_g.transpose(1, 2, 3, 0)                       # [g, i, k, bb]
        W5 = np.zeros((N_GROUPS, GROUP, K_CURR, GROUP, NUM_BASES),
                      dtype=np.float32)
        W5[:, ii, :, ii, :] = w_g.transpose(1, 0, 2, 3)
        w_list.append(np.ascontiguousarray(
            W5.reshape(N_GROUPS, 128, 32).transpose(1, 0, 2).reshape(128, -1)
        ).astype(BF16))

    repl = {"z_cat": z_cat, "cm": CM, "sel": SEL}
    per_core = {
        "idx": np.concatenate(idx_list, axis=0),
        "w": np.concatenate(w_list, axis=0),
    }
    return repl, per_core


def _make_runner(nc, n_cores):
    """Cached jitted SPMD dispatcher (see module docstring)."""
    import jax
    import jax.numpy as jnp
    from jax.sharding import Mesh, PartitionSpec as P, NamedSharding
    from jax.experimental.shard_map import shard_map
    from concourse.bass2jax import (
        _bass_exec_p,
        install_neuronx_cc_hook,
        partition_id_tensor,
    )

    install_neuronx_cc_hook()
    assert nc.dbg_addr is None or not nc.dbg_callbacks

    partition_name = (
        nc.partition_id_tensor.name if nc.partition_id_tensor else None
    )
    in_names, out_names, out_avals, zero_shapes = [], [], [], []
    for alloc in nc.m.functions[0].allocations:
        if not isinstance(alloc, mybir.MemoryLocationSet):
            continue
        name = alloc.memorylocations[0].name
        if alloc.kind == "ExternalInput":
            if name != partition_name:
                in_names.append(name)
        elif alloc.kind == "ExternalOutput":
            out_names.append(name)
            shape = tuple(alloc.tensor_shape)
            dtype = mybir.dt.np(alloc.dtype)
            out_avals.append(jax.core.ShapedArray(shape, dtype))
            zero_shapes.append((shape, dtype))
    n_params = len(in_names)
    n_outs = len(out_avals)
    all_in_names = list(in_names) + list(out_names)
    if partition_name is not None:
        all_in_names.append(partition_name)
    donate = tuple(range(n_params, n_params + n_outs))

    def _body(*args):
        operands = list(args)
        if partition_name is not None:
            operands.append(partition_id_tensor())
        outs = _bass_exec_p.bind(
            *operands,
            out_avals=tuple(out_avals),
            in_names=tuple(all_in_names),
            out_names=tuple(out_names),
            lowering_input_output_aliases=(),
            sim_require_finite=True,
            sim_require_nnan=True,
            nc=nc,
        )
        return tuple(outs)

    devices = jax.devices()[:n_cores]
    assert len(devices) == n_cores
    mesh = Mesh(np.asarray(devices), ("core",))

    in_specs = tuple(
        P() if name in _REPL_NAMES else P("core") for name in in_names
    ) + (P("core"),) * n_outs
    out_specs = (P("core"),) * n_outs
    sharded = jax.jit(
        shard_map(_body, mesh=mesh, in_specs=in_specs,
                  out_specs=out_specs, check_rep=False),
        donate_argnums=donate,
        keep_unused=True,
    )

    repl_names = [n for n in in_names if n in _REPL_NAMES]

    def _gather_body(*xs):
        return tuple(
            jax.lax.all_gather(x, "core", axis=0, tiled=True) for x in xs
        )

    gatherer = jax.jit(
        shard_map(_gather_body, mesh=mesh,
                  in_specs=(P("core"),) * len(repl_names),
                  out_specs=(P(),) * len(repl_names),
                  check_rep=False)
    )

    zeros_fns = [
        jax.jit(
            lambda s=s, d=d: jnp.zeros((n_cores * s[0], *s[1:]), d),
            out_shardings=NamedSharding(mesh, P("core")),
        )
        for (s, d) in zero_shapes
    ]

    def run(repl, per_core):
        repl_dev = dict(zip(repl_names, gatherer(*[repl[n] for n in repl_names])))
        args = [
            repl_dev[n] if n in repl_dev else per_core[n] for n in in_names
        ]
        zeros = [zf() for zf in zeros_fns]
        out_arrs = sharded(*args, *zeros)
        outs = {}
        for i, name in enumerate(out_names):
            s, d = zero_shapes[i]
            outs[name] = np.asarray(out_arrs[i]).reshape(n_cores, *s)
        return outs

    return run


def kernel(z, neighbor_indices, adjacency, basis_weights, channel_coeffs):
    assert z.shape == (B, N, T, C), z.shape
    key = "prog"
    if key not in _compiled_cache:
        nc = _build_program(reps=1)
        _compiled_cache[key] = (nc, _make_runner(nc, N_CORES))
    nc, run = _compiled_cache[key]

    repl, per_core = _prep_common(z, neighbor_indices, adjacency,
                                  basis_weights, channel_coeffs)
    out_cat = run(repl, per_core)["out"]                      # [8, 256, F] bf16
    out = (out_cat.astype(np.float32)
           .reshape(N, B, T, C).transpose(1, 0, 2, 3))
    return np.ascontiguousarray(out)
